# revision 6
# baseline (speedup 1.0000x reference)
"""Sliding-window GQA attention (T=4096, DIM=2048, H=16, KVH=4, D=128, W=1024)
as an 8-core SPMD Trainium2 Bass/Tile kernel — v3.

v3 vs v2: consolidated big DMAs (one per weight matrix / x span, sprayed
across all 16 queues; descriptor-gen per DMA is ~0.6us on the issuing
sequencer so fewer DMAs = less serialization), DMA issuance spread over
sync/scalar/gpsimd sequencers and issued early, M_ORDER interleaves
small/large exp tiles so ACT keeps ahead of the PE, wo prefetched during
attention so the O-projection tail runs dense.

Sharding: sequence-parallel. Core c owns queries [512c, 512c+512) and
recomputes K/V for its sliding window (1536 kv slots, zero-padded before
position 0). No collectives.
"""

import math
import os
import sys

import numpy as np


def _ensure_paths():
    for p in (
        "/root/.axon_site",
        "/root/.axon_site/_ro/trn_rl_repo",
        "/root/.axon_site/_ro/pypackages",
        "/opt/trn_rl_repo",
        "/opt/pypackages",
    ):
        if os.path.isdir(p) and p not in sys.path:
            sys.path.append(p)


try:
    import concourse.bass as bass  # noqa: F401
except ImportError:
    _ensure_paths()

import ml_dtypes
import concourse.bass as bass
import concourse.mybir as mybir
import concourse.tile as tile
from concourse import bacc
from concourse.bass_utils import run_bass_kernel_spmd

# ---------------------------------------------------------------- constants
N_CORES = 8
T = 4096
DIM = 2048
H = 16
KVH = 4
D = 128
WIN = 1024
ROPE_BASE = 10000.0

TQ = T // N_CORES          # 512 queries per core
TKV = TQ + WIN             # 1536 kv slots per core
NMT = TKV // 128           # 12 kv tiles of 128
NCC = DIM // 128           # 16 contraction chunks
SCALE = 1.0 / math.sqrt(D)
GQ = H // KVH              # 4 q heads per kv head

F32 = mybir.dt.float32
BF16 = mybir.dt.bfloat16
BF = ml_dtypes.bfloat16

# per kv-tile m: (qlo, qhi) span of local queries it can interact with
SPANS = {
    0: (0, 128), 1: (0, 256), 2: (0, 384), 3: (0, 512),
    4: (0, 512), 5: (0, 512), 6: (0, 512), 7: (0, 512),
    8: (0, 512), 9: (128, 512), 10: (256, 512), 11: (384, 512),
}
# per kv-tile m: (which_ext_mask, mask_lo, mask_hi, q_lo, q_hi) in absolute
# q coords; ext masks are [128, 256] with the all-zero region baked in so
# each tile needs exactly one multiply
MASKS = {
    0: ("B", 0, 128, 0, 128), 1: ("B", 0, 128, 128, 256),
    2: ("B", 0, 128, 256, 384), 3: ("B", 0, 128, 384, 512),
    4: None, 5: None, 6: None, 7: None,
    8: ("A", 128, 256, 0, 128), 9: ("A", 128, 256, 128, 256),
    10: ("A", 128, 256, 256, 384), 11: ("A", 128, 256, 384, 512),
}
# PSUM accumulation order: m=4 first (full-width span -> start=True clears
# the whole Y/den bank), m=11 last (stop=True). Narrow boundary tiles are
# interleaved between full-width ones so the ACT exp stream stays ahead.
M_ORDER = [4, 0, 5, 1, 6, 2, 7, 3, 8, 9, 10, 11]
LOOKAHEAD = 2


# ---------------------------------------------------------------- device code
_NC_CACHE = None


def _build():
    global _NC_CACHE
    if _NC_CACHE is not None:
        return _NC_CACHE

    nc = bacc.Bacc("TRN2", target_bir_lowering=False, debug=False,
                   num_devices=N_CORES)

    # big-DMA layouts: each SBUF tile is one contiguous DRAM block
    xq = nc.dram_tensor("xq", [128, NCC * 512], BF16, kind="ExternalInput").ap()
    xkv = nc.dram_tensor("xkv", [3 * 128, NCC * 512], BF16,
                         kind="ExternalInput").ap()
    wq = nc.dram_tensor("wq", [8 * 128, NCC * 256], BF16,
                        kind="ExternalInput").ap()
    wk = nc.dram_tensor("wk", [128, NCC * 512], BF16, kind="ExternalInput").ap()
    wv = nc.dram_tensor("wv", [128, NCC * 512], BF16, kind="ExternalInput").ap()
    wo = nc.dram_tensor("wo", [8 * 128, NCC * 256], BF16,
                        kind="ExternalInput").ap()
    cosq = nc.dram_tensor("cosq", [D, TQ], BF16, kind="ExternalInput").ap()
    sinq = nc.dram_tensor("sinq", [D, TQ], BF16, kind="ExternalInput").ap()
    cosk = nc.dram_tensor("cosk", [D, TKV], BF16, kind="ExternalInput").ap()
    sink = nc.dram_tensor("sink", [D, TKV], BF16, kind="ExternalInput").ap()
    kbias = nc.dram_tensor("kbias", [128, NMT], F32, kind="ExternalInput").ap()
    maskB = nc.dram_tensor("maskB", [128, 256], BF16, kind="ExternalInput").ap()
    maskA = nc.dram_tensor("maskA", [128, 256], BF16, kind="ExternalInput").ap()
    rotp = nc.dram_tensor("rotp", [128, 128], BF16, kind="ExternalInput").ap()
    ones = nc.dram_tensor("ones", [128, 128], BF16, kind="ExternalInput").ap()
    outT = nc.dram_tensor("outT", [DIM, TQ], F32, kind="ExternalOutput").ap()

    mask_dram = {"maskB": maskB, "maskA": maskA}

    with tile.TileContext(nc) as tc:
        _emit(nc, tc, xq, xkv, wq, wk, wv, wo, cosq, sinq, cosk, sink,
              kbias, mask_dram, rotp, ones, outT)

    nc.compile()
    _NC_CACHE = nc
    return nc


def _emit(nc, tc, xq, xkv, wq, wk, wv, wo, cosq, sinq, cosk, sink,
          kbias, mask_dram, rotp, ones, outT):
    from contextlib import ExitStack

    ctx = ExitStack()
    with ctx:
        # ---- SBUF pools
        consts = ctx.enter_context(tc.tile_pool(name="consts", bufs=1))
        xkvp = ctx.enter_context(tc.tile_pool(name="xkvp", bufs=2))
        wqp = ctx.enter_context(tc.tile_pool(name="wqp", bufs=2))
        wop = ctx.enter_context(tc.tile_pool(name="wop", bufs=4))
        ktp = ctx.enter_context(tc.tile_pool(name="ktp", bufs=KVH))
        vp = ctx.enter_context(tc.tile_pool(name="vp", bufs=NMT))
        qtp = ctx.enter_context(tc.tile_pool(name="qtp", bufs=4))
        ytp = ctx.enter_context(tc.tile_pool(name="ytp", bufs=H))
        pp = ctx.enter_context(tc.tile_pool(name="pp", bufs=5))
        tmp = ctx.enter_context(tc.tile_pool(name="tmp", bufs=2))
        t12 = ctx.enter_context(tc.tile_pool(name="t12", bufs=4))
        fin = ctx.enter_context(tc.tile_pool(name="fin", bufs=2))
        # ---- PSUM pools (8 banks total)
        pS1 = ctx.enter_context(tc.tile_pool(name="pS1", bufs=2, space="PSUM"))
        pS2 = ctx.enter_context(tc.tile_pool(name="pS2", bufs=1, space="PSUM"))
        pY = ctx.enter_context(tc.tile_pool(name="pY", bufs=2, space="PSUM"))
        pQ = ctx.enter_context(tc.tile_pool(name="pQ", bufs=1, space="PSUM"))
        pD = ctx.enter_context(tc.tile_pool(name="pD", bufs=2, space="PSUM"))

        Exp = mybir.ActivationFunctionType.Exp

        # ---- input DMAs, spread across sequencers in consume order.
        # Each tensor split in 4 parts: sprays better across DMA queues and
        # lets the PE start on part 0 while the rest streams.
        def dma4(eng, dst, src, parts=4):
            n = dst.shape[-1]
            step = n // parts
            for i in range(0, n, step):
                eng.dma_start(dst[:, i:i + step], src[:, i:i + step])

        # gpsimd: phase A weights, in need order (8 parts up front so the
        # first chunks land fast)
        wk_sb = consts.tile([128, NCC * 512], BF16, tag="wk", name="wk_sb")
        dma4(nc.gpsimd, wk_sb[:], wk[:], parts=16)
        wv_sb = consts.tile([128, NCC * 512], BF16, tag="wv", name="wv_sb")
        dma4(nc.gpsimd, wv_sb[:], wv[:], parts=8)

        # scalar: x spans (idle until attention)
        xs_tiles = {}

        def fetch_span(s, parts=4):
            xs = xkvp.tile([128, NCC * 512], BF16, tag="xkv", name=f"xkv{s}")
            dma4(nc.scalar, xs[:], xkv[s * 128:(s + 1) * 128, :], parts=parts)
            xs_tiles[s] = xs

        fetch_span(0, parts=16)
        fetch_span(1)

        # sync: small consts first (rope tables needed early), then x
        # queries and q-proj weights (needed at phase B)
        def cload(ap, shape, dtype, tag):
            t = consts.tile(shape, dtype, tag=tag, name=tag)
            nc.sync.dma_start(t[:], ap[:])
            return t

        rotp_sb = cload(rotp, [128, 128], BF16, "rotp")
        ones_sb = cload(ones, [128, 128], BF16, "ones")
        kbias_sb = cload(kbias, [128, NMT], F32, "kbias")
        cosk_sb = cload(cosk, [D, TKV], BF16, "cosk")
        sink_sb = cload(sink, [D, TKV], BF16, "sink")
        cosq_sb = cload(cosq, [D, TQ], BF16, "cosq")
        sinq_sb = cload(sinq, [D, TQ], BF16, "sinq")
        mask_sb = {
            "B": cload(mask_dram["maskB"], [128, 256], BF16, "maskB"),
            "A": cload(mask_dram["maskA"], [128, 256], BF16, "maskA"),
        }

        xq_sb = consts.tile([128, NCC * 512], BF16, tag="xq", name="xq_sb")
        dma4(nc.sync, xq_sb[:], xq[:])
        wq_tiles = {}

        def fetch_wq(p_):
            if p_ in wq_tiles or p_ >= H // 2:
                return
            wqt = wqp.tile([128, NCC * 256], BF16, tag="wq", name=f"wq{p_}")
            dma4(nc.sync, wqt[:], wq[p_ * 128:(p_ + 1) * 128, :], parts=2)
            wq_tiles[p_] = wqt

        fetch_wq(0)
        fetch_wq(1)

        Copy = mybir.ActivationFunctionType.Copy

        def rope(src_ps, sin_sl, cos_sl, dst_ap, width):
            """dst = src*cos + rot_half(src)*sin  (dst bf16)."""
            s_sb = tmp.tile([128, 512], BF16, tag="ropesb", name="ropesb")
            nc.scalar.activation(s_sb[:, :width], src_ps[:, :width], Copy)
            r_ps = pD.tile([128, 512], F32, tag="pD", name="ropeps")
            nc.tensor.matmul(r_ps[:, :width], rotp_sb[:], s_sb[:, :width],
                             start=True, stop=True)
            t1 = t12.tile([128, 512], F32, tag="t12", name="ropet1")
            nc.vector.tensor_mul(t1[:, :width], r_ps[:, :width], sin_sl)
            t2 = t12.tile([128, 512], F32, tag="t12", name="ropet2")
            nc.vector.tensor_mul(t2[:, :width], src_ps[:, :width], cos_sl)
            nc.vector.tensor_add(dst_ap, t1[:, :width], t2[:, :width])

        # ---- phase B helpers (defined early: proj(0)/proj(1) are drained
        # inside phase A so their ropes overlap the span-2 V matmuls)
        qts = {}

        def proj_gen(h):
            """Yields after each PE instruction; Q proj + rope for head h."""
            p_, j = h // 2, h % 2
            fetch_wq(p_ + 1)  # prefetch next pair's weights
            qps = pQ.tile([128, 512], F32, tag="pQ", name=f"qps{h}")
            for c in range(NCC):
                nc.tensor.matmul(
                    qps[:],
                    wq_tiles[p_][:, c * 256 + j * 128:c * 256 + (j + 1) * 128],
                    xq_sb[:, c * 512:(c + 1) * 512],
                    start=(c == 0), stop=(c == NCC - 1))
                yield
            if j == 1 and p_ - 1 in wq_tiles:
                del wq_tiles[p_ - 1]
            qtj = qtp.tile([128, TQ], BF16, tag="qt", name=f"qt{h}")
            rope(qps, sinq_sb[:], cosq_sb[:], qtj[:], TQ)
            qts[h] = qtj
            yield

        def drain(it):
            if it is not None:
                for _ in it:
                    pass

        # ---- phase A: K^T (RoPE'd) and V over 3 spans of 512 kv slots
        kt_sb = [ktp.tile([128, TKV], BF16, tag="kt", name=f"kt{g}")
                 for g in range(KVH)]
        v_sb = [vp.tile([128, 512], BF16, tag="v", name=f"v{m}")
                for m in range(NMT)]

        for s in range(3):
            if s + 1 < 3:
                fetch_span(s + 1)
            xs = xs_tiles.pop(s)
            # K^T projection: g-outer, 16 accumulating MMs per g, then rope
            for g in range(KVH):
                kps = pS1.tile([128, 512], F32, tag="pS1", name=f"kps{s}_{g}")
                for c in range(NCC):
                    nc.tensor.matmul(
                        kps[:],
                        wk_sb[:, c * 512 + g * 128:c * 512 + (g + 1) * 128],
                        xs[:, c * 512:(c + 1) * 512],
                        start=(c == 0), stop=(c == NCC - 1))
                rope(kps, sink_sb[:, s * 512:(s + 1) * 512],
                     cosk_sb[:, s * 512:(s + 1) * 512],
                     kt_sb[g][:, s * 512:(s + 1) * 512], 512)

            if s == 2:
                # heads 0/1 projected here: their ropes run on DVE while
                # the PE streams the span-2 V matmuls below
                drain(proj_gen(0))
                drain(proj_gen(1))

            # V projection (natural layout): tt-outer
            for tt in range(4):
                vps = pY.tile([128, 512], F32, tag="pY", name=f"vps{s}_{tt}")
                for c in range(NCC):
                    nc.tensor.matmul(
                        vps[:],
                        xs[:, c * 512 + tt * 128:c * 512 + (tt + 1) * 128],
                        wv_sb[:, c * 512:(c + 1) * 512],
                        start=(c == 0), stop=(c == NCC - 1))
                nc.scalar.activation(v_sb[4 * s + tt][:], vps[:], Copy)

        # wo prefetch. Pairs 0-3 ride in the xkv pool's slots: the ring's
        # reuse dependency delays their DMAs until phase A frees the space,
        # so they can't steal HBM bandwidth from the phase A loads even
        # though gpsimd issues descriptors early. Pairs 4-7 are issued from
        # scalar between heads (queue-ordered behind the x spans).
        wo_tiles = {}
        for i in range(2):
            wob = xkvp.tile([128, NCC * 512], BF16, tag="xkv",
                            name=f"wobig{i}")
            for k in range(2):
                np_ = 2 * i + k
                dma4(nc.gpsimd, wob[:, k * 4096:(k + 1) * 4096],
                     wo[np_ * 128:(np_ + 1) * 128, :])
                wo_tiles[np_] = (wob, k * 4096)

        def fetch_wo(np_, eng):
            if np_ in wo_tiles or np_ >= NCC // 2:
                return
            wot = wop.tile([128, NCC * 256], BF16, tag="wo", name=f"wo{np_}")
            dma4(eng, wot[:], wo[np_ * 128:(np_ + 1) * 128, :])
            wo_tiles[np_] = (wot, 0)

        # ---- phases B+C: per-head Q proj (as PE filler) + attention
        yt_sb = [ytp.tile([128, TQ], BF16, tag="yt", name=f"yt{h}")
                 for h in range(H)]

        def emit_attn(h, filler=None):
            g = h // GQ
            qt = qts.pop(h)
            yps = pY.tile([128, TQ], F32, tag="pY", name=f"yps{h}")
            dps = pD.tile([128, TQ], F32, tag="pD", name=f"dps{h}")
            p_tiles = {}

            def emit_yden(mi):
                m = M_ORDER[mi]
                qlo, qhi = SPANS[m]
                w = qhi - qlo
                p = p_tiles.pop(mi)
                first = mi == 0
                last = mi == len(M_ORDER) - 1
                nc.tensor.matmul(yps[:, qlo:qhi],
                                 v_sb[m][:, g * 128:(g + 1) * 128],
                                 p[:, :w], start=first, stop=last)
                nc.tensor.matmul(dps[:, qlo:qhi], ones_sb[:], p[:, :w],
                                 start=first, stop=last)

            for mi, m in enumerate(M_ORDER):
                qlo, qhi = SPANS[m]
                w = qhi - qlo
                pool = pS2 if mi % 3 == 2 else pS1
                sps = pool.tile([128, 512], F32, tag=pool.name,
                                name=f"sps{h}_{m}")
                nc.tensor.matmul(sps[:, :w],
                                 kt_sb[g][:, m * 128:(m + 1) * 128],
                                 qt[:, qlo:qhi], start=True, stop=True)
                p = pp.tile([128, 512], BF16, tag="p", name=f"p{h}_{m}")
                nc.scalar.activation(p[:, :w], sps[:, :w], Exp,
                                     bias=kbias_sb[:, m:m + 1], scale=SCALE)
                mk = MASKS[m]
                if mk is not None:
                    which, mlo, mhi, lo, hi = mk
                    nc.vector.tensor_mul(p[:, lo - qlo:hi - qlo],
                                         p[:, lo - qlo:hi - qlo],
                                         mask_sb[which][:, mlo:mhi])
                p_tiles[mi] = p
                if filler is not None:
                    next(filler, None)
                if mi >= LOOKAHEAD:
                    emit_yden(mi - LOOKAHEAD)
                if filler is not None and mi % 2 == 0 and mi < 8:
                    next(filler, None)
            for mi in range(len(M_ORDER) - LOOKAHEAD, len(M_ORDER)):
                # keep a couple of proj MMs between the tail Y/dens so the
                # PE has cover while the last exps drain
                if filler is not None:
                    next(filler, None)
                    next(filler, None)
                emit_yden(mi)

            # normalize (den is in [1, ~1e4]: safe for the fast reciprocal)
            rcp = fin.tile([128, TQ], F32, tag="rcp", name=f"rcp{h}")
            nc.vector.reciprocal_approx_fast(rcp[:], dps[:])
            nc.vector.tensor_mul(yt_sb[h][:], yps[:], rcp[:])

        # head pipeline: proj(0)/proj(1) were drained in phase A;
        # proj(h+2) emits as filler inside ATTN(h)
        for h in range(H):
            filler = proj_gen(h + 2) if h + 2 < H else None
            emit_attn(h, filler)
            drain(filler)
            if h in (1, 3, 5, 7):
                fetch_wo(4 + h // 2, nc.scalar)

        # ---- phase D: O^T projection in e-tile pairs
        for n0 in range(0, NCC, 2):
            np_ = n0 // 2
            wot, base = wo_tiles[np_]
            opair = [pS1.tile([128, 512], F32, tag="pS1", name=f"ops{n0}_0"),
                     pS2.tile([128, 512], F32, tag="pS2", name=f"ops{n0}_1")]
            for hh in range(H):
                for j in range(2):
                    nc.tensor.matmul(
                        opair[j][:],
                        wot[:, base + hh * 256 + j * 128:
                            base + hh * 256 + (j + 1) * 128],
                        yt_sb[hh][:],
                        start=(hh == 0), stop=(hh == H - 1))
            for j in range(2):
                osb = fin.tile([128, TQ], F32, tag="osb", name=f"osb{n0}_{j}")
                nc.scalar.activation(osb[:], opair[j][:], Copy)
                nc.sync.dma_start(outT[(n0 + j) * 128:(n0 + j + 1) * 128, :],
                                  osb[:])


# ---------------------------------------------------------------- host side
def _chunkmajor(a, rows, cols):
    """[n*rows, cols] -> [rows, n*cols] with block c at cols [c*cols:...]."""
    n = a.shape[0] // rows
    return np.ascontiguousarray(
        a.reshape(n, rows, cols).transpose(1, 0, 2).reshape(rows, n * cols))


def _host_inputs(x, Wq, Wk, Wv, Wo):
    x = np.asarray(x, dtype=np.float32).reshape(T, DIM)

    inv_freq = 1.0 / (ROPE_BASE ** (np.arange(0, D, 2, dtype=np.float64) / D))
    dfreq = np.concatenate([inv_freq, inv_freq])  # [128] per-dim freq

    # wq/wo: per pair p: [2048, 256] -> [128, 16*256]; stacked -> [1024, 4096]
    wq_b = np.concatenate(
        [_chunkmajor(np.asarray(Wq, np.float32)[:, p * 256:(p + 1) * 256],
                     128, 256) for p in range(8)], axis=0).astype(BF)
    wo_b = np.concatenate(
        [_chunkmajor(np.asarray(Wo, np.float32)[:, p * 256:(p + 1) * 256],
                     128, 256) for p in range(8)], axis=0).astype(BF)
    wk_b = _chunkmajor(np.asarray(Wk, np.float32), 128, 512).astype(BF)
    wv_b = _chunkmajor(np.asarray(Wv, np.float32), 128, 512).astype(BF)

    u = np.arange(128)[:, None]
    maskB = np.concatenate(  # [triangle | zeros]
        [(np.arange(128)[None, :] < u), np.zeros((128, 128), bool)],
        axis=1).astype(BF)
    maskA = np.concatenate(  # [zeros | triangle]
        [np.zeros((128, 128), bool), (u <= np.arange(128)[None, :])],
        axis=1).astype(BF)

    rotp = np.zeros((128, 128), np.float32)
    d = np.arange(128)
    rotp[(d + 64) % 128, d] = 1.0  # out[d] = in[(d+64)%128]

    ones = np.ones((128, 128), BF)

    in_maps = []
    for c in range(N_CORES):
        qs = c * TQ
        xq = x[qs:qs + TQ]                      # [512, 2048]
        xkv = np.zeros((TKV, DIM), np.float32)  # [1536, 2048]
        lo = qs - WIN
        src_lo = max(0, lo)
        xkv[src_lo - lo:TKV] = x[src_lo:qs + TQ]

        pos_q = np.arange(qs, qs + TQ, dtype=np.float64)
        pos_k = np.arange(lo, qs + TQ, dtype=np.float64)
        angq = dfreq[:, None] * pos_q[None, :]  # [128, 512]
        angk = dfreq[:, None] * pos_k[None, :]  # [128, 1536]
        sgn = np.where(np.arange(D) < D // 2, -1.0, 1.0)[:, None]

        kb = np.zeros((128, NMT), np.float32)
        for m in range(NMT):
            t_abs = 128 * m + np.arange(128)
            kb[:, m] = np.where(t_abs < WIN - qs, -30.0, 0.0)

        # x spans: [1536, 2048]^T per span -> [3*128, 16*512]
        xkvT = np.concatenate(
            [_chunkmajor(np.ascontiguousarray(
                xkv[s * 512:(s + 1) * 512].T), 128, 512) for s in range(3)],
            axis=0)

        in_maps.append({
            "xq": _chunkmajor(np.ascontiguousarray(xq.T), 128, 512).astype(BF),
            "xkv": xkvT.astype(BF),
            "wq": wq_b, "wk": wk_b, "wv": wv_b, "wo": wo_b,
            "cosq": np.cos(angq).astype(BF),
            "sinq": (sgn * np.sin(angq)).astype(BF),
            "cosk": np.cos(angk).astype(BF),
            "sink": (sgn * np.sin(angk)).astype(BF),
            "kbias": kb,
            "maskB": maskB, "maskA": maskA,
            "rotp": rotp.astype(BF),
            "ones": ones,
        })
    return in_maps


def kernel(x, Wq, Wk, Wv, Wo, _trace=False, _trace_kwargs=None):
    nc = _build()
    in_maps = _host_inputs(x, Wq, Wk, Wv, Wo)
    res = run_bass_kernel_spmd(nc, in_maps, core_ids=list(range(N_CORES)),
                               trace=_trace, **(_trace_kwargs or {}))
    out = np.empty((1, T, DIM), np.float32)
    for c in range(N_CORES):
        out[0, c * TQ:(c + 1) * TQ, :] = res.results[c]["outT"].T
    if _trace:
        kernel.last_results = res
    return out


# revision 7
# speedup vs baseline: 1.0074x; 1.0074x over previous
"""Sliding-window GQA attention (T=4096, DIM=2048, H=16, KVH=4, D=128, W=1024)
as an 8-core SPMD Trainium2 Bass/Tile kernel — v3.

v3 vs v2: consolidated big DMAs (one per weight matrix / x span, sprayed
across all 16 queues; descriptor-gen per DMA is ~0.6us on the issuing
sequencer so fewer DMAs = less serialization), DMA issuance spread over
sync/scalar/gpsimd sequencers and issued early, M_ORDER interleaves
small/large exp tiles so ACT keeps ahead of the PE, wo prefetched during
attention so the O-projection tail runs dense.

Sharding: sequence-parallel. Core c owns queries [512c, 512c+512) and
recomputes K/V for its sliding window (1536 kv slots, zero-padded before
position 0). No collectives.
"""

import math
import os
import sys

import numpy as np


def _ensure_paths():
    for p in (
        "/root/.axon_site",
        "/root/.axon_site/_ro/trn_rl_repo",
        "/root/.axon_site/_ro/pypackages",
        "/opt/trn_rl_repo",
        "/opt/pypackages",
    ):
        if os.path.isdir(p) and p not in sys.path:
            sys.path.append(p)


try:
    import concourse.bass as bass  # noqa: F401
except ImportError:
    _ensure_paths()

import ml_dtypes
import concourse.bass as bass
import concourse.mybir as mybir
import concourse.tile as tile
from concourse import bacc
from concourse.bass_utils import run_bass_kernel_spmd

# ---------------------------------------------------------------- constants
N_CORES = 8
T = 4096
DIM = 2048
H = 16
KVH = 4
D = 128
WIN = 1024
ROPE_BASE = 10000.0

TQ = T // N_CORES          # 512 queries per core
TKV = TQ + WIN             # 1536 kv slots per core
NMT = TKV // 128           # 12 kv tiles of 128
NCC = DIM // 128           # 16 contraction chunks
SCALE = 1.0 / math.sqrt(D)
GQ = H // KVH              # 4 q heads per kv head

F32 = mybir.dt.float32
BF16 = mybir.dt.bfloat16
BF = ml_dtypes.bfloat16

# per kv-tile m: (qlo, qhi) span of local queries it can interact with
SPANS = {
    0: (0, 128), 1: (0, 256), 2: (0, 384), 3: (0, 512),
    4: (0, 512), 5: (0, 512), 6: (0, 512), 7: (0, 512),
    8: (0, 512), 9: (128, 512), 10: (256, 512), 11: (384, 512),
}
# per kv-tile m: (which_ext_mask, mask_lo, mask_hi, q_lo, q_hi) in absolute
# q coords; ext masks are [128, 256] with the all-zero region baked in so
# each tile needs exactly one multiply
MASKS = {
    0: ("B", 0, 128, 0, 128), 1: ("B", 0, 128, 128, 256),
    2: ("B", 0, 128, 256, 384), 3: ("B", 0, 128, 384, 512),
    4: None, 5: None, 6: None, 7: None,
    8: ("A", 128, 256, 0, 128), 9: ("A", 128, 256, 128, 256),
    10: ("A", 128, 256, 256, 384), 11: ("A", 128, 256, 384, 512),
}
# PSUM accumulation order: m=4 first (full-width span -> start=True clears
# the whole Y/den bank), m=11 last (stop=True). Narrow boundary tiles are
# interleaved between full-width ones so the ACT exp stream stays ahead.
M_ORDER = [4, 0, 5, 1, 6, 2, 7, 3, 8, 9, 10, 11]
LOOKAHEAD = 2


# ---------------------------------------------------------------- device code
_NC_CACHE = None


def _build():
    global _NC_CACHE
    if _NC_CACHE is not None:
        return _NC_CACHE

    nc = bacc.Bacc("TRN2", target_bir_lowering=False, debug=False,
                   num_devices=N_CORES)

    # big-DMA layouts: each SBUF tile is one contiguous DRAM block
    xq = nc.dram_tensor("xq", [128, NCC * 512], BF16, kind="ExternalInput").ap()
    xkv = nc.dram_tensor("xkv", [3 * 128, NCC * 512], BF16,
                         kind="ExternalInput").ap()
    wq = nc.dram_tensor("wq", [8 * 128, NCC * 256], BF16,
                        kind="ExternalInput").ap()
    wk = nc.dram_tensor("wk", [128, NCC * 512], BF16, kind="ExternalInput").ap()
    wv = nc.dram_tensor("wv", [128, NCC * 512], BF16, kind="ExternalInput").ap()
    wo = nc.dram_tensor("wo", [8 * 128, NCC * 256], BF16,
                        kind="ExternalInput").ap()
    cosq = nc.dram_tensor("cosq", [D, TQ], BF16, kind="ExternalInput").ap()
    sinq = nc.dram_tensor("sinq", [D, TQ], BF16, kind="ExternalInput").ap()
    cosk = nc.dram_tensor("cosk", [D, TKV], BF16, kind="ExternalInput").ap()
    sink = nc.dram_tensor("sink", [D, TKV], BF16, kind="ExternalInput").ap()
    kbias = nc.dram_tensor("kbias", [128, NMT], F32, kind="ExternalInput").ap()
    maskB = nc.dram_tensor("maskB", [128, 256], BF16, kind="ExternalInput").ap()
    maskA = nc.dram_tensor("maskA", [128, 256], BF16, kind="ExternalInput").ap()
    rotp = nc.dram_tensor("rotp", [128, 128], BF16, kind="ExternalInput").ap()
    ones = nc.dram_tensor("ones", [128, 128], BF16, kind="ExternalInput").ap()
    outT = nc.dram_tensor("outT", [DIM, TQ], F32, kind="ExternalOutput").ap()

    mask_dram = {"maskB": maskB, "maskA": maskA}

    with tile.TileContext(nc) as tc:
        _emit(nc, tc, xq, xkv, wq, wk, wv, wo, cosq, sinq, cosk, sink,
              kbias, mask_dram, rotp, ones, outT)

    nc.compile()
    _NC_CACHE = nc
    return nc


def _emit(nc, tc, xq, xkv, wq, wk, wv, wo, cosq, sinq, cosk, sink,
          kbias, mask_dram, rotp, ones, outT):
    from contextlib import ExitStack

    ctx = ExitStack()
    with ctx:
        # ---- SBUF pools
        consts = ctx.enter_context(tc.tile_pool(name="consts", bufs=1))
        xkvp = ctx.enter_context(tc.tile_pool(name="xkvp", bufs=2))
        wqp = ctx.enter_context(tc.tile_pool(name="wqp", bufs=2))
        wop = ctx.enter_context(tc.tile_pool(name="wop", bufs=4))
        ktp = ctx.enter_context(tc.tile_pool(name="ktp", bufs=KVH))
        vp = ctx.enter_context(tc.tile_pool(name="vp", bufs=NMT))
        qtp = ctx.enter_context(tc.tile_pool(name="qtp", bufs=4))
        ytp = ctx.enter_context(tc.tile_pool(name="ytp", bufs=H))
        pp = ctx.enter_context(tc.tile_pool(name="pp", bufs=5))
        tmp = ctx.enter_context(tc.tile_pool(name="tmp", bufs=2))
        t12 = ctx.enter_context(tc.tile_pool(name="t12", bufs=4))
        fin = ctx.enter_context(tc.tile_pool(name="fin", bufs=2))
        # ---- PSUM pools (8 banks total)
        pS1 = ctx.enter_context(tc.tile_pool(name="pS1", bufs=2, space="PSUM"))
        pS2 = ctx.enter_context(tc.tile_pool(name="pS2", bufs=1, space="PSUM"))
        pY = ctx.enter_context(tc.tile_pool(name="pY", bufs=2, space="PSUM"))
        pQ = ctx.enter_context(tc.tile_pool(name="pQ", bufs=1, space="PSUM"))
        pD = ctx.enter_context(tc.tile_pool(name="pD", bufs=2, space="PSUM"))

        Exp = mybir.ActivationFunctionType.Exp

        # ---- input DMAs, spread across sequencers in consume order.
        # Each tensor split in 4 parts: sprays better across DMA queues and
        # lets the PE start on part 0 while the rest streams.
        def dma4(eng, dst, src, parts=4):
            n = dst.shape[-1]
            step = n // parts
            for i in range(0, n, step):
                eng.dma_start(dst[:, i:i + step], src[:, i:i + step])

        # gpsimd: phase A weights, wk/wv parts interleaved so wv starts
        # landing before the K passes finish (V follows K by ~14us)
        wk_sb = consts.tile([128, NCC * 512], BF16, tag="wk", name="wk_sb")
        wv_sb = consts.tile([128, NCC * 512], BF16, tag="wv", name="wv_sb")
        for i in range(8):
            st = i * 1024
            nc.gpsimd.dma_start(wk_sb[:, st:st + 1024], wk[:, st:st + 1024])
            nc.gpsimd.dma_start(wv_sb[:, st:st + 1024], wv[:, st:st + 1024])

        # scalar: x spans (idle until attention)
        xs_tiles = {}

        def fetch_span(s, parts=4):
            if s in xs_tiles:  # guard: a duplicate fetch burns a ring slot
                return         # gated on the whole previous span's reads
            xs = xkvp.tile([128, NCC * 512], BF16, tag="xkv", name=f"xkv{s}")
            dma4(nc.scalar, xs[:], xkv[s * 128:(s + 1) * 128, :], parts=parts)
            xs_tiles[s] = xs

        fetch_span(0, parts=16)
        fetch_span(1)

        # sync: small consts first (rope tables needed early), then x
        # queries and q-proj weights (needed at phase B)
        def cload(ap, shape, dtype, tag):
            t = consts.tile(shape, dtype, tag=tag, name=tag)
            nc.sync.dma_start(t[:], ap[:])
            return t

        rotp_sb = cload(rotp, [128, 128], BF16, "rotp")
        ones_sb = cload(ones, [128, 128], BF16, "ones")
        kbias_sb = cload(kbias, [128, NMT], F32, "kbias")
        cosk_sb = cload(cosk, [D, TKV], BF16, "cosk")
        sink_sb = cload(sink, [D, TKV], BF16, "sink")
        cosq_sb = cload(cosq, [D, TQ], BF16, "cosq")
        sinq_sb = cload(sinq, [D, TQ], BF16, "sinq")
        mask_sb = {
            "B": cload(mask_dram["maskB"], [128, 256], BF16, "maskB"),
            "A": cload(mask_dram["maskA"], [128, 256], BF16, "maskA"),
        }

        xq_sb = consts.tile([128, NCC * 512], BF16, tag="xq", name="xq_sb")
        dma4(nc.sync, xq_sb[:], xq[:])
        wq_tiles = {}

        def fetch_wq(p_):
            if p_ in wq_tiles or p_ >= H // 2:
                return
            wqt = wqp.tile([128, NCC * 256], BF16, tag="wq", name=f"wq{p_}")
            dma4(nc.sync, wqt[:], wq[p_ * 128:(p_ + 1) * 128, :], parts=2)
            wq_tiles[p_] = wqt

        fetch_wq(0)
        fetch_wq(1)

        Copy = mybir.ActivationFunctionType.Copy

        def rope(src_ps, sin_sl, cos_sl, dst_ap, width):
            """dst = src*cos + rot_half(src)*sin  (dst bf16)."""
            s_sb = tmp.tile([128, 512], BF16, tag="ropesb", name="ropesb")
            nc.scalar.activation(s_sb[:, :width], src_ps[:, :width], Copy)
            r_ps = pD.tile([128, 512], F32, tag="pD", name="ropeps")
            nc.tensor.matmul(r_ps[:, :width], rotp_sb[:], s_sb[:, :width],
                             start=True, stop=True)
            t1 = t12.tile([128, 512], F32, tag="t12", name="ropet1")
            nc.vector.tensor_mul(t1[:, :width], r_ps[:, :width], sin_sl)
            t2 = t12.tile([128, 512], F32, tag="t12", name="ropet2")
            nc.vector.tensor_mul(t2[:, :width], src_ps[:, :width], cos_sl)
            nc.vector.tensor_add(dst_ap, t1[:, :width], t2[:, :width])

        # ---- phase B helpers (defined early: proj(0)/proj(1) are drained
        # inside phase A so their ropes overlap the span-2 V matmuls)
        qts = {}

        def proj_gen(h):
            """Yields after each PE instruction; Q proj + rope for head h."""
            p_, j = h // 2, h % 2
            fetch_wq(p_ + 1)  # prefetch next pair's weights
            qps = pQ.tile([128, 512], F32, tag="pQ", name=f"qps{h}")
            for c in range(NCC):
                nc.tensor.matmul(
                    qps[:],
                    wq_tiles[p_][:, c * 256 + j * 128:c * 256 + (j + 1) * 128],
                    xq_sb[:, c * 512:(c + 1) * 512],
                    start=(c == 0), stop=(c == NCC - 1))
                yield
            if j == 1 and p_ - 1 in wq_tiles:
                del wq_tiles[p_ - 1]
            qtj = qtp.tile([128, TQ], BF16, tag="qt", name=f"qt{h}")
            rope(qps, sinq_sb[:], cosq_sb[:], qtj[:], TQ)
            qts[h] = qtj
            yield

        def drain(it):
            if it is not None:
                for _ in it:
                    pass

        # ---- phase A: K^T (RoPE'd) and V over 3 spans of 512 kv slots
        kt_sb = [ktp.tile([128, TKV], BF16, tag="kt", name=f"kt{g}")
                 for g in range(KVH)]
        v_sb = [vp.tile([128, 512], BF16, tag="v", name=f"v{m}")
                for m in range(NMT)]

        for s in range(3):
            if s + 1 < 3:
                fetch_span(s + 1)
            xs = xs_tiles.pop(s)
            # K^T projection: g-outer, 16 accumulating MMs per g, then rope
            for g in range(KVH):
                kps = pS1.tile([128, 512], F32, tag="pS1", name=f"kps{s}_{g}")
                for c in range(NCC):
                    nc.tensor.matmul(
                        kps[:],
                        wk_sb[:, c * 512 + g * 128:c * 512 + (g + 1) * 128],
                        xs[:, c * 512:(c + 1) * 512],
                        start=(c == 0), stop=(c == NCC - 1))
                rope(kps, sink_sb[:, s * 512:(s + 1) * 512],
                     cosk_sb[:, s * 512:(s + 1) * 512],
                     kt_sb[g][:, s * 512:(s + 1) * 512], 512)

            if s == 2:
                # heads 0/1 projected here: their ropes run on DVE while
                # the PE streams the span-2 V matmuls below
                drain(proj_gen(0))
                drain(proj_gen(1))

            # V projection (natural layout): tt-outer
            for tt in range(4):
                vps = pY.tile([128, 512], F32, tag="pY", name=f"vps{s}_{tt}")
                for c in range(NCC):
                    nc.tensor.matmul(
                        vps[:],
                        xs[:, c * 512 + tt * 128:c * 512 + (tt + 1) * 128],
                        wv_sb[:, c * 512:(c + 1) * 512],
                        start=(c == 0), stop=(c == NCC - 1))
                nc.scalar.activation(v_sb[4 * s + tt][:], vps[:], Copy)

        # wo prefetch. Pairs 0-3 ride in the xkv pool's slots: the ring's
        # reuse dependency delays their DMAs until phase A frees the space,
        # so they can't steal HBM bandwidth from the phase A loads even
        # though gpsimd issues descriptors early. Pairs 4-7 are issued from
        # scalar between heads (queue-ordered behind the x spans).
        wo_tiles = {}
        for i in range(2):
            wob = xkvp.tile([128, NCC * 512], BF16, tag="xkv",
                            name=f"wobig{i}")
            for k in range(2):
                np_ = 2 * i + k
                dma4(nc.gpsimd, wob[:, k * 4096:(k + 1) * 4096],
                     wo[np_ * 128:(np_ + 1) * 128, :])
                wo_tiles[np_] = (wob, k * 4096)

        def fetch_wo(np_, eng):
            if np_ in wo_tiles or np_ >= NCC // 2:
                return
            wot = wop.tile([128, NCC * 256], BF16, tag="wo", name=f"wo{np_}")
            dma4(eng, wot[:], wo[np_ * 128:(np_ + 1) * 128, :])
            wo_tiles[np_] = (wot, 0)

        # ---- phases B+C: per-head Q proj (as PE filler) + attention
        yt_sb = [ytp.tile([128, TQ], BF16, tag="yt", name=f"yt{h}")
                 for h in range(H)]

        def emit_attn(h, filler=None):
            g = h // GQ
            qt = qts.pop(h)
            yps = pY.tile([128, TQ], F32, tag="pY", name=f"yps{h}")
            dps = pD.tile([128, TQ], F32, tag="pD", name=f"dps{h}")
            p_tiles = {}

            def emit_yden(mi):
                m = M_ORDER[mi]
                qlo, qhi = SPANS[m]
                w = qhi - qlo
                p = p_tiles.pop(mi)
                first = mi == 0
                last = mi == len(M_ORDER) - 1
                nc.tensor.matmul(yps[:, qlo:qhi],
                                 v_sb[m][:, g * 128:(g + 1) * 128],
                                 p[:, :w], start=first, stop=last)
                nc.tensor.matmul(dps[:, qlo:qhi], ones_sb[:], p[:, :w],
                                 start=first, stop=last)

            for mi, m in enumerate(M_ORDER):
                qlo, qhi = SPANS[m]
                w = qhi - qlo
                pool = pS2 if mi % 3 == 2 else pS1
                sps = pool.tile([128, 512], F32, tag=pool.name,
                                name=f"sps{h}_{m}")
                nc.tensor.matmul(sps[:, :w],
                                 kt_sb[g][:, m * 128:(m + 1) * 128],
                                 qt[:, qlo:qhi], start=True, stop=True)
                p = pp.tile([128, 512], BF16, tag="p", name=f"p{h}_{m}")
                nc.scalar.activation(p[:, :w], sps[:, :w], Exp,
                                     bias=kbias_sb[:, m:m + 1], scale=SCALE)
                mk = MASKS[m]
                if mk is not None:
                    which, mlo, mhi, lo, hi = mk
                    nc.vector.tensor_mul(p[:, lo - qlo:hi - qlo],
                                         p[:, lo - qlo:hi - qlo],
                                         mask_sb[which][:, mlo:mhi])
                p_tiles[mi] = p
                if filler is not None:
                    next(filler, None)
                if mi >= LOOKAHEAD:
                    emit_yden(mi - LOOKAHEAD)
                if filler is not None and mi % 2 == 0 and mi < 8:
                    next(filler, None)
            for mi in range(len(M_ORDER) - LOOKAHEAD, len(M_ORDER)):
                # keep a couple of proj MMs between the tail Y/dens so the
                # PE has cover while the last exps drain
                if filler is not None:
                    next(filler, None)
                    next(filler, None)
                emit_yden(mi)

            # normalize (den is in [1, ~1e4]: safe for the fast reciprocal)
            rcp = fin.tile([128, TQ], F32, tag="rcp", name=f"rcp{h}")
            nc.vector.reciprocal_approx_fast(rcp[:], dps[:])
            nc.vector.tensor_mul(yt_sb[h][:], yps[:], rcp[:])

        # head pipeline: proj(0)/proj(1) were drained in phase A;
        # proj(h+2) emits as filler inside ATTN(h)
        for h in range(H):
            filler = proj_gen(h + 2) if h + 2 < H else None
            emit_attn(h, filler)
            drain(filler)
            if h in (1, 3, 5, 7):
                fetch_wo(4 + h // 2, nc.scalar)

        # ---- phase D: O^T projection in e-tile pairs
        for n0 in range(0, NCC, 2):
            np_ = n0 // 2
            wot, base = wo_tiles[np_]
            opair = [pS1.tile([128, 512], F32, tag="pS1", name=f"ops{n0}_0"),
                     pS2.tile([128, 512], F32, tag="pS2", name=f"ops{n0}_1")]
            for hh in range(H):
                for j in range(2):
                    nc.tensor.matmul(
                        opair[j][:],
                        wot[:, base + hh * 256 + j * 128:
                            base + hh * 256 + (j + 1) * 128],
                        yt_sb[hh][:],
                        start=(hh == 0), stop=(hh == H - 1))
            for j in range(2):
                osb = fin.tile([128, TQ], F32, tag="osb", name=f"osb{n0}_{j}")
                nc.scalar.activation(osb[:], opair[j][:], Copy)
                nc.sync.dma_start(outT[(n0 + j) * 128:(n0 + j + 1) * 128, :],
                                  osb[:])


# ---------------------------------------------------------------- host side
def _chunkmajor(a, rows, cols):
    """[n*rows, cols] -> [rows, n*cols] with block c at cols [c*cols:...]."""
    n = a.shape[0] // rows
    return np.ascontiguousarray(
        a.reshape(n, rows, cols).transpose(1, 0, 2).reshape(rows, n * cols))


def _host_inputs(x, Wq, Wk, Wv, Wo):
    x = np.asarray(x, dtype=np.float32).reshape(T, DIM)

    inv_freq = 1.0 / (ROPE_BASE ** (np.arange(0, D, 2, dtype=np.float64) / D))
    dfreq = np.concatenate([inv_freq, inv_freq])  # [128] per-dim freq

    # wq/wo: per pair p: [2048, 256] -> [128, 16*256]; stacked -> [1024, 4096]
    wq_b = np.concatenate(
        [_chunkmajor(np.asarray(Wq, np.float32)[:, p * 256:(p + 1) * 256],
                     128, 256) for p in range(8)], axis=0).astype(BF)
    wo_b = np.concatenate(
        [_chunkmajor(np.asarray(Wo, np.float32)[:, p * 256:(p + 1) * 256],
                     128, 256) for p in range(8)], axis=0).astype(BF)
    wk_b = _chunkmajor(np.asarray(Wk, np.float32), 128, 512).astype(BF)
    wv_b = _chunkmajor(np.asarray(Wv, np.float32), 128, 512).astype(BF)

    u = np.arange(128)[:, None]
    maskB = np.concatenate(  # [triangle | zeros]
        [(np.arange(128)[None, :] < u), np.zeros((128, 128), bool)],
        axis=1).astype(BF)
    maskA = np.concatenate(  # [zeros | triangle]
        [np.zeros((128, 128), bool), (u <= np.arange(128)[None, :])],
        axis=1).astype(BF)

    rotp = np.zeros((128, 128), np.float32)
    d = np.arange(128)
    rotp[(d + 64) % 128, d] = 1.0  # out[d] = in[(d+64)%128]

    ones = np.ones((128, 128), BF)

    in_maps = []
    for c in range(N_CORES):
        qs = c * TQ
        xq = x[qs:qs + TQ]                      # [512, 2048]
        xkv = np.zeros((TKV, DIM), np.float32)  # [1536, 2048]
        lo = qs - WIN
        src_lo = max(0, lo)
        xkv[src_lo - lo:TKV] = x[src_lo:qs + TQ]

        pos_q = np.arange(qs, qs + TQ, dtype=np.float64)
        pos_k = np.arange(lo, qs + TQ, dtype=np.float64)
        angq = dfreq[:, None] * pos_q[None, :]  # [128, 512]
        angk = dfreq[:, None] * pos_k[None, :]  # [128, 1536]
        sgn = np.where(np.arange(D) < D // 2, -1.0, 1.0)[:, None]

        kb = np.zeros((128, NMT), np.float32)
        for m in range(NMT):
            t_abs = 128 * m + np.arange(128)
            kb[:, m] = np.where(t_abs < WIN - qs, -30.0, 0.0)

        # x spans: [1536, 2048]^T per span -> [3*128, 16*512]
        xkvT = np.concatenate(
            [_chunkmajor(np.ascontiguousarray(
                xkv[s * 512:(s + 1) * 512].T), 128, 512) for s in range(3)],
            axis=0)

        in_maps.append({
            "xq": _chunkmajor(np.ascontiguousarray(xq.T), 128, 512).astype(BF),
            "xkv": xkvT.astype(BF),
            "wq": wq_b, "wk": wk_b, "wv": wv_b, "wo": wo_b,
            "cosq": np.cos(angq).astype(BF),
            "sinq": (sgn * np.sin(angq)).astype(BF),
            "cosk": np.cos(angk).astype(BF),
            "sink": (sgn * np.sin(angk)).astype(BF),
            "kbias": kb,
            "maskB": maskB, "maskA": maskA,
            "rotp": rotp.astype(BF),
            "ones": ones,
        })
    return in_maps


def kernel(x, Wq, Wk, Wv, Wo, _trace=False, _trace_kwargs=None):
    nc = _build()
    in_maps = _host_inputs(x, Wq, Wk, Wv, Wo)
    res = run_bass_kernel_spmd(nc, in_maps, core_ids=list(range(N_CORES)),
                               trace=_trace, **(_trace_kwargs or {}))
    out = np.empty((1, T, DIM), np.float32)
    for c in range(N_CORES):
        out[0, c * TQ:(c + 1) * TQ, :] = res.results[c]["outT"].T
    if _trace:
        kernel.last_results = res
    return out
